# revision 19
# baseline (speedup 1.0000x reference)
"""Trainium2 Bass kernel for nn_CAM_6949257085456.

Data-parallel over batch: 8 cores x 64 samples (1024 activation rows each).
Per core the rows are processed as 5 pipeline slices (256/256/256/192/64):

  stream(s):  large fully-contiguous x DMAs feed the folded vis encoder
              matmul  visT = (W_enc2 @ W_red) @ x  (196 k-chunks, N=cs)
  attn(s-1):  the attention/branch stage of the previous slice is emitted
              interleaved between the stream's matmul groups so the PE stays
              dense-busy (keeps the HAM clock-gate at 2.4 GHz) while DMA paces.
              The last slice is small (64 rows) so the un-overlapped tail is
              one attention tile.

Attention restructure vs the obvious per-sample loop: 4 samples live in one
128-partition tile at 32-row pitch, and the per-sample 16/32-contraction
linears (W_aff / W_a / W_ca / W_h) are folded into block-diagonal 128-row
weights on the host, so each stage is a single full-K matmul (or a
tile_position row-strip matmul for the data*data attention product).

Host-side algebraic folds (exact in fp32):
  - vis path: X @ W_red.T @ W_enc2.T == X @ (W_enc2 @ W_red).T
  - regressors: feats@Wv1.T@Wv2.T == feats @ (Wv2@Wv1).T
Everything fed to the chip is bf16 (fp32 PSUM accumulation).
"""
import sys

if "/opt/trn_rl_repo" not in sys.path:
    sys.path.insert(0, "/opt/trn_rl_repo")

import numpy as np
import ml_dtypes

import concourse.bacc as bacc
import concourse.bass as bass
import concourse.mybir as mybir
import concourse.tile as tile
from concourse import bass_utils

BF16 = mybir.dt.bfloat16
F32 = mybir.dt.float32
AF = mybir.ActivationFunctionType

B, T, DA, DV, DH = 512, 16, 512, 25088, 128
NCORES = 8
S = B // NCORES            # samples per core (64)
R = S * T                  # rows per core (1024)
KC = DV // 128             # contraction chunks (196)
NG = 14                    # chunk groups (14 chunks of 128 each)
SCALE = 1.0 / 16.0         # 1/sqrt(256)

SLICES = [256, 256, 256, 256]
OFFS = [0, 256, 512, 768]
NSL = len(SLICES)
# attention-tile emission quanta per tile count of the previous slice
QA = {4: {2: 0, 5: 1, 8: 2, 11: 3}, 3: {2: 0, 5: 1, 8: 2}, 1: {2: 0}}

_CACHE = {}


def _build():
    import os

    STAGE = int(os.environ.get("KSTAGE", "0"))
    nc = bacc.Bacc("TRN2", target_bir_lowering=False, debug=False)

    xg_d = [
        nc.dram_tensor(f"xg{s}", [NG, 128, NG, cs], BF16, kind="ExternalInput")
        for s, cs in enumerate(SLICES)
    ]
    wg = nc.dram_tensor("wg", [NG, 128, NG, 128], BF16, kind="ExternalInput")
    f1g = nc.dram_tensor("f1g", [128, 4, R], BF16, kind="ExternalInput")
    wenc1 = nc.dram_tensor("wenc1", [128, 4, 128], BF16, kind="ExternalInput")
    b1 = nc.dram_tensor("b1", [DH, 1], F32, kind="ExternalInput")
    b2 = nc.dram_tensor("b2", [DH, 1], F32, kind="ExternalInput")
    waff = nc.dram_tensor("waff", [128, 2, 128], BF16, kind="ExternalInput")
    wca = nc.dram_tensor("wca", [128, 8, 128], BF16, kind="ExternalInput")
    wa = nc.dram_tensor("wa", [128, 128], BF16, kind="ExternalInput")
    wh = nc.dram_tensor("wh", [128, 2, 64], BF16, kind="ExternalInput")
    wreg = nc.dram_tensor("wreg", [128, 2, 2], BF16, kind="ExternalInput")
    creg = nc.dram_tensor("creg", [2, 1], F32, kind="ExternalInput")
    ident = nc.dram_tensor("ident", [128, 128], BF16, kind="ExternalInput")

    outs = nc.dram_tensor("outs", [2, R], F32, kind="ExternalOutput")

    from contextlib import ExitStack

    with tile.TileContext(nc) as tc:
        with ExitStack() as stack:
            ec = stack.enter_context
            cpool = ec(tc.tile_pool(name="const", bufs=1))
            wpool = ec(tc.tile_pool(name="wred", bufs=14))
            xpool = ec(tc.tile_pool(name="xin", bufs=6))
            actpool = ec(tc.tile_pool(name="acts", bufs=4))
            rtpool = ec(tc.tile_pool(name="rowsT", bufs=4))
            gsbpool = ec(tc.tile_pool(name="gsb", bufs=2))
            attsbpool = ec(tc.tile_pool(name="attsb", bufs=5))
            htsbpool = ec(tc.tile_pool(name="htsb", bufs=2))
            outsbpool = ec(tc.tile_pool(name="outsb", bufs=4))
            encpool = ec(tc.tile_pool(name="enc_ps", bufs=2, space="PSUM"))
            attpool = ec(tc.tile_pool(name="att_ps", bufs=2, space="PSUM"))
            midpool = ec(tc.tile_pool(name="mid_ps", bufs=2, space="PSUM"))
            outpool = ec(tc.tile_pool(name="out_ps", bufs=2, space="PSUM"))

            # ---- constants / weights (loaded once; f1 first — the slice-0
            # aud matmuls are the earliest consumer of a scalar-queued DMA)
            f1_sb = cpool.tile([128, 4, R], BF16, name="f1_sb")
            nc.scalar.dma_start(f1_sb[:], f1g[:])
            wenc1_sb = cpool.tile([128, 4, 128], BF16, name="wenc1_sb")
            nc.scalar.dma_start(wenc1_sb[:], wenc1[:])
            ident_sb = cpool.tile([128, 128], BF16, name="ident_sb")
            nc.scalar.dma_start(ident_sb[:], ident[:])
            b1_sb = cpool.tile([DH, 1], F32, name="b1_sb")
            nc.scalar.dma_start(b1_sb[:], b1[:])
            b2_sb = cpool.tile([DH, 1], F32, name="b2_sb")
            nc.scalar.dma_start(b2_sb[:], b2[:])
            creg_sb = cpool.tile([2, 1], F32, name="creg_sb")
            nc.scalar.dma_start(creg_sb[:], creg[:])
            waff_sb = cpool.tile([128, 2, 128], BF16, name="waff_sb")
            nc.scalar.dma_start(waff_sb[:], waff[:])
            wca_sb = cpool.tile([128, 8, 128], BF16, name="wca_sb")
            nc.scalar.dma_start(wca_sb[:], wca[:])
            wa_sb = cpool.tile([128, 128], BF16, name="wa_sb")
            nc.scalar.dma_start(wa_sb[:], wa[:])
            wh_sb = cpool.tile([128, 2, 64], BF16, name="wh_sb")
            nc.scalar.dma_start(wh_sb[:], wh[:])
            wreg_sb = cpool.tile([128, 2, 2], BF16, name="wreg_sb")
            nc.scalar.dma_start(wreg_sb[:], wreg[:])
            final_sb = cpool.tile([2, R], F32, name="final_sb")

            # ---- PE warmup: dummy matmuls to trip the HAM clock gate ----
            for i in range(32):
                wp = attpool.tile([128, 128], F32, tag="attps", name=f"warm{i}")
                nc.tensor.matmul(wp[:], ident_sb[:], ident_sb[:],
                                 start=True, stop=True)
            # preload the scalar-engine tanh table
            warmt_sb = cpool.tile([128, 4], BF16, name="warmt_sb", tag="warmt")
            nc.scalar.activation(warmt_sb[:], ident_sb[:, 0:4], AF.Tanh)

            # long-lived avf tiles, zeroed once: rows 16..31 of each 32-row
            # sample group are never written again, so the block-diagonal
            # weights always multiply zeros (not uninitialized bf16).
            avf_glob = []
            for i in range(6):
                av = cpool.tile([128, 256], BF16, tag=f"avfg{i}",
                                name=f"avfg{i}")
                nc.vector.memset(av[:], 0.0)
                avf_glob.append(av)

            w_tiles = []
            state = {}  # per-slice live tiles

            def emit_avf_tr(sp, blk):
                """PE-transpose 128 activation rows of slice sp into rt."""
                st = state[sp]
                cs = SLICES[sp]
                bw = min(128, cs - 128 * blk)
                rt = rtpool.tile([128, 256], BF16, tag="rt",
                                 name=f"rt{sp}_{blk}")
                st["rt"][blk] = rt
                for bi, src in enumerate((st["audT"], st["visT"])):
                    tr_ps = midpool.tile([128, 128], BF16, tag="mid",
                                         name=f"tr{sp}_{blk}_{bi}")
                    nc.tensor.transpose(
                        tr_ps[0:bw, :],
                        src[:, 128 * blk:128 * blk + bw],
                        ident_sb[:],
                    )
                    nc.vector.tensor_copy(rt[0:bw, 128 * bi:128 * bi + 128],
                                          tr_ps[0:bw, :])

            def emit_regroup(sp, a, eng=None):
                """Partition-regroup 4 samples of rt into avf tile a."""
                st = state[sp]
                rt = st["rt"][a // 2]
                for q in range(4):
                    m = 4 * (a % 2) + q
                    (eng or nc.scalar).dma_start(
                        st["avf"][a][32 * q:32 * q + 16, :],
                        rt[16 * m:16 * m + 16, :],
                    )

            def emit_attn_tile(sp, a):
                """Attention/branch stage for 4 samples (avf tile a, slice sp)."""
                st = state[sp]
                avf_t = st["avf"][a]
                g_ps = midpool.tile([128, 256], F32, tag="mid", name=f"g{sp}_{a}")
                for bi in range(2):
                    nc.tensor.matmul(
                        g_ps[:, 128 * bi:128 * bi + 128],
                        waff_sb[:, bi, :],
                        avf_t[:, 128 * bi:128 * bi + 128],
                        start=True, stop=True,
                    )
                g_sb = gsbpool.tile([128, 256], BF16, tag="gsb",
                                    name=f"gsb{sp}_{a}")
                nc.vector.tensor_copy(g_sb[:], g_ps[:])

                att_sbs = []
                for q in range(4):
                    att_ps = attpool.tile([128, 512], F32, tag="attps",
                                          name=f"att{sp}_{a}_{q}")
                    for jh in range(2):
                        nc.tensor.matmul(
                            att_ps[:, 256 * jh:256 * jh + 256],
                            avf_t[32 * q:32 * q + 16, 128 * jh:128 * jh + 128],
                            g_sb[32 * q:32 * q + 16, :],
                            start=True, stop=True,
                            tile_position=(32 * q, 0),
                        )
                    asb = attsbpool.tile([128, 512], BF16, tag="attsb",
                                         name=f"asb{sp}_{a}_{q}")
                    nc.scalar.activation(asb[:], att_ps[:], AF.Tanh, scale=SCALE)
                    att_sbs.append(asb)

                ht_ps = midpool.tile([128, 256], F32, tag="mid",
                                     name=f"ht{sp}_{a}")
                for q in range(4):
                    for jh in range(2):
                        nc.tensor.matmul(
                            ht_ps[:],
                            wca_sb[:, 2 * q + jh, :],
                            att_sbs[q][:, 256 * jh:256 * jh + 256],
                            start=(q == 0 and jh == 0), stop=False,
                        )
                nc.tensor.matmul(ht_ps[:], wa_sb[:], avf_t[:],
                                 start=False, stop=True)
                ht_sb = htsbpool.tile([128, 256], BF16, tag="htsb",
                                      name=f"htsb{sp}_{a}")
                nc.vector.tensor_relu(ht_sb[:], ht_ps[:])

                out_ps = outpool.tile([128, 2, 64], F32, tag="o2",
                                      name=f"o{sp}_{a}")
                for bi in range(2):
                    nc.tensor.matmul(
                        out_ps[:, bi, :],
                        ht_sb[:, 128 * bi:128 * bi + 128],
                        wh_sb[:, bi, :],
                        start=True, stop=True,
                    )
                for bi, act in enumerate((st["audT"], st["visT"])):
                    if STAGE == 1:
                        nc.vector.tensor_copy(
                            st["outsb"][bi][:, 64 * a:64 * a + 64],
                            act[:, 64 * a:64 * a + 64],
                        )
                    else:
                        nc.vector.tensor_add(
                            st["outsb"][bi][:, 64 * a:64 * a + 64],
                            out_ps[:, bi, :],
                            act[:, 64 * a:64 * a + 64],
                        )

            def emit_regressor(sp):
                st = state[sp]
                cs, off = SLICES[sp], OFFS[sp]
                out2_ps = outpool.tile([2, 256], F32, tag="o2",
                                       name=f"out2{sp}")
                nc.tensor.matmul(out2_ps[:, 0:cs], wreg_sb[:, 0, :],
                                 st["outsb"][0][:, 0:cs], start=True, stop=False)
                nc.tensor.matmul(out2_ps[:, 0:cs], wreg_sb[:, 1, :],
                                 st["outsb"][1][:, 0:cs], start=False, stop=True)
                nc.scalar.activation(
                    final_sb[:, off:off + cs], out2_ps[:, 0:cs],
                    AF.Identity, bias=creg_sb[:],
                )
                nc.scalar.dma_start(outs[:, off:off + cs],
                                    final_sb[:, off:off + cs])

            filler_n = [0]

            def emit_filler(n):
                # dummy matmuls that bridge PE-idle windows so the HAM
                # clock gate stays at 2.4 GHz across slice boundaries
                for _ in range(n):
                    i = filler_n[0]
                    filler_n[0] += 1
                    fp = attpool.tile([128, 128], F32, tag="attps",
                                      name=f"fill{i}")
                    nc.tensor.matmul(fp[:], ident_sb[:], ident_sb[:],
                                     start=True, stop=True)

            def attn_quantum(sp, g):
                if sp < 0:
                    return
                nt = SLICES[sp] // 64
                nblk = (nt + 1) // 2
                if g < nblk:
                    emit_avf_tr(sp, g)
                if g < nt:
                    emit_regroup(sp, g)
                qa = QA[nt]
                if g in qa:
                    emit_attn_tile(sp, qa[g])
                elif g == 12:
                    emit_regressor(sp)

            tile_base = 0
            for s, cs in enumerate(SLICES):
                off = OFFS[s]
                enc_ps = encpool.tile([128, 512], F32, tag="enc",
                                      name=f"enc{s}")
                for g in range(NG):
                    if s == 0:
                        wt = wpool.tile([128, NG, 128], BF16, tag="w",
                                        name=f"wt{g}")
                        nc.sync.dma_start(wt[:], wg[g])
                        w_tiles.append(wt)
                    xk = xpool.tile([128, NG, cs], BF16, tag="xk",
                                    name=f"xk{s}_{g}")
                    nc.sync.dma_start(xk[:], xg_d[s][g])
                    for j in range(NG):
                        nc.tensor.matmul(
                            enc_ps[:, 0:cs],
                            w_tiles[g][:, j, :],
                            xk[:, j, :],
                            start=(g == 0 and j == 0),
                            stop=(g == NG - 1 and j == NG - 1),
                        )
                    if g == 4:
                        # aud encoder rides inside the enc accumulation group
                        # (start bit already consumed by vis chunk 0)
                        for c in range(4):
                            nc.tensor.matmul(
                                enc_ps[:, 256:256 + cs],
                                wenc1_sb[:, c, :],
                                f1_sb[:, c, off:off + cs],
                                start=False, stop=False,
                            )
                    attn_quantum(s - 1, g)

                # slice boundary: move encoder outputs to SBUF (bias fused)
                audT = actpool.tile([128, 256], BF16, tag="act",
                                    name=f"audT{s}")
                nc.scalar.activation(audT[:, 0:cs], enc_ps[:, 256:256 + cs],
                                     AF.Identity, bias=b1_sb[:])
                visT = actpool.tile([128, 256], BF16, tag="act",
                                    name=f"visT{s}")
                nc.scalar.activation(visT[:, 0:cs], enc_ps[:, 0:cs],
                                     AF.Identity, bias=b2_sb[:])
                nt = cs // 64
                avf = [avf_glob[(tile_base + a) % 6] for a in range(nt)]
                tile_base += nt
                outsb = [
                    outsbpool.tile([128, 256], BF16, tag="outsb",
                                   name=f"os{s}_{bi}")
                    for bi in range(2)
                ]
                state[s] = dict(audT=audT, visT=visT, avf=avf, outsb=outsb,
                                rt=[None, None])

            # tail: attention for the last slice (sync queue is free now)
            sp = NSL - 1
            nt_t = SLICES[sp] // 64
            for blk in range((nt_t + 1) // 2):
                emit_avf_tr(sp, blk)
            for a in range(nt_t):
                emit_regroup(sp, a, eng=nc.sync)
            # single-tile PE filler: bridges the regroup-latency idle window
            # so the HAM clock gate stays at 2.4 GHz for the tail attention
            fill_ps = attpool.tile([128, 128], F32, tag="attps",
                                   name="tailfill")
            for _ in range(20):
                nc.tensor.matmul(fill_ps[:], ident_sb[:], ident_sb[:],
                                 start=True, stop=True)
            for a in range(nt_t):
                emit_attn_tile(sp, a)
            emit_regressor(sp)

    nc.compile()
    return nc


def _prep_shared(inputs):
    f32 = np.float32
    bf = ml_dtypes.bfloat16
    W_enc1 = np.asarray(inputs["W_enc1"], f32)
    W_enc2 = np.asarray(inputs["W_enc2"], f32)
    W_red = np.asarray(inputs["W_red"], f32)
    W2r = W_enc2 @ W_red                                    # [128, 25088]
    b2v = W_enc2 @ np.asarray(inputs["b_red"], f32) + np.asarray(inputs["b_enc2"], f32)
    wv = (np.asarray(inputs["Wv2"], f32) @ np.asarray(inputs["Wv1"], f32))[0]
    cv = float((np.asarray(inputs["Wv2"], f32) @ np.asarray(inputs["bv1"], f32)
                + np.asarray(inputs["bv2"], f32))[0])
    wa_ = (np.asarray(inputs["Wa2"], f32) @ np.asarray(inputs["Wa1"], f32))[0]
    ca = float((np.asarray(inputs["Wa2"], f32) @ np.asarray(inputs["ba1"], f32)
                + np.asarray(inputs["ba2"], f32))[0])

    W_affa = np.asarray(inputs["W_affa"], f32)
    W_affv = np.asarray(inputs["W_affv"], f32)
    W_a = np.asarray(inputs["W_a"], f32)
    W_ca = np.asarray(inputs["W_ca"], f32)
    W_ha = np.asarray(inputs["W_ha"], f32)
    W_hv = np.asarray(inputs["W_hv"], f32)

    waff_bd = np.zeros((2, 128, 128), f32)
    for q in range(4):
        waff_bd[0, 32 * q:32 * q + 16, 32 * q:32 * q + 16] = W_affa.T
        waff_bd[1, 32 * q:32 * q + 16, 32 * q:32 * q + 16] = W_affv.T
    wca_bd = np.zeros((4, 2, 128, 128), f32)
    for q in range(4):
        for jh in range(2):
            wca_bd[q, jh, :, 32 * q:32 * q + 32] = W_ca[:, 128 * jh:128 * jh + 128].T
    wa_bd = np.zeros((128, 128), f32)
    for q in range(4):
        wa_bd[32 * q:32 * q + 16, 32 * q:32 * q + 32] = W_a.T
    wh_bd = np.zeros((2, 128, 64), f32)
    for q in range(4):
        wh_bd[0, 32 * q:32 * q + 32, 16 * q:16 * q + 16] = W_ha.T
        wh_bd[1, 32 * q:32 * q + 32, 16 * q:16 * q + 16] = W_hv.T

    wg = np.ascontiguousarray(
        W2r.T.reshape(NG, NG, 128, 128).transpose(0, 2, 1, 3)
    ).astype(bf)
    wenc1g = np.ascontiguousarray(
        W_enc1.T.reshape(4, 128, 128).transpose(1, 0, 2)
    ).astype(bf)

    shared = {
        "wg": wg,
        "wenc1": wenc1g,
        "b1": np.asarray(inputs["b_enc1"], f32).reshape(128, 1),
        "b2": b2v.reshape(128, 1),
        "waff": np.ascontiguousarray(waff_bd.transpose(1, 0, 2)).astype(bf),
        "wca": np.ascontiguousarray(
            wca_bd.reshape(8, 128, 128).transpose(1, 0, 2)
        ).astype(bf),
        "wa": wa_bd.astype(bf),
        "wh": np.ascontiguousarray(wh_bd.transpose(1, 0, 2)).astype(bf),
        "wreg": np.ascontiguousarray(
            np.stack(
                [np.stack([wv[:128], wa_[:128]], 1),
                 np.stack([wv[128:], wa_[128:]], 1)], 1
            )
        ).astype(bf),
        "creg": np.array([[cv], [ca]], f32),
        "ident": np.eye(128, dtype=f32).astype(bf),
    }
    return shared


def kernel(**inputs):
    if "nc" not in _CACHE:
        _CACHE["nc"] = _build()
    nc = _CACHE["nc"]

    bf = ml_dtypes.bfloat16
    shared = _prep_shared(inputs)

    f1 = np.asarray(inputs["f1_norm"], np.float32).reshape(B * T, DA)
    f2 = np.asarray(inputs["f2_norm"], np.float32).reshape(B * T, DV)

    in_maps = []
    for c in range(NCORES):
        rs = slice(c * R, (c + 1) * R)
        m = dict(shared)
        xT4 = f2[rs].T.reshape(NG, NG, 128, R)          # [g, j, p, r]
        for s, cs in enumerate(SLICES):
            off = OFFS[s]
            m[f"xg{s}"] = np.ascontiguousarray(
                xT4[:, :, :, off:off + cs].transpose(0, 2, 1, 3)
            ).astype(bf)
        m["f1g"] = np.ascontiguousarray(
            f1[rs].T.reshape(4, 128, R).transpose(1, 0, 2)
        ).astype(bf)
        in_maps.append(m)

    import os

    res = bass_utils.run_bass_kernel_spmd(
        nc,
        in_maps,
        core_ids=list(range(NCORES)),
        trace=bool(os.environ.get("KERNEL_TRACE")),
    )
    _CACHE["last_results"] = res

    vouts = np.concatenate(
        [r["outs"][0].reshape(S, T) for r in res.results], axis=0
    ).astype(np.float32)
    aouts = np.concatenate(
        [r["outs"][1].reshape(S, T) for r in res.results], axis=0
    ).astype(np.float32)
    return vouts, aouts


# revision 20
# speedup vs baseline: 1.0176x; 1.0176x over previous
"""Trainium2 Bass kernel for nn_CAM_6949257085456.

Data-parallel over batch: 8 cores x 64 samples (1024 activation rows each).
Per core the rows are processed as 5 pipeline slices (256/256/256/192/64):

  stream(s):  large fully-contiguous x DMAs feed the folded vis encoder
              matmul  visT = (W_enc2 @ W_red) @ x  (196 k-chunks, N=cs)
  attn(s-1):  the attention/branch stage of the previous slice is emitted
              interleaved between the stream's matmul groups so the PE stays
              dense-busy (keeps the HAM clock-gate at 2.4 GHz) while DMA paces.
              The last slice is small (64 rows) so the un-overlapped tail is
              one attention tile.

Attention restructure vs the obvious per-sample loop: 4 samples live in one
128-partition tile at 32-row pitch, and the per-sample 16/32-contraction
linears (W_aff / W_a / W_ca / W_h) are folded into block-diagonal 128-row
weights on the host, so each stage is a single full-K matmul (or a
tile_position row-strip matmul for the data*data attention product).

Host-side algebraic folds (exact in fp32):
  - vis path: X @ W_red.T @ W_enc2.T == X @ (W_enc2 @ W_red).T
  - regressors: feats@Wv1.T@Wv2.T == feats @ (Wv2@Wv1).T
Everything fed to the chip is bf16 (fp32 PSUM accumulation).
"""
import sys

if "/opt/trn_rl_repo" not in sys.path:
    sys.path.insert(0, "/opt/trn_rl_repo")

import numpy as np
import ml_dtypes

import concourse.bacc as bacc
import concourse.bass as bass
import concourse.mybir as mybir
import concourse.tile as tile
from concourse import bass_utils

BF16 = mybir.dt.bfloat16
F32 = mybir.dt.float32
AF = mybir.ActivationFunctionType

B, T, DA, DV, DH = 512, 16, 512, 25088, 128
NCORES = 8
S = B // NCORES            # samples per core (64)
R = S * T                  # rows per core (1024)
KC = DV // 128             # contraction chunks (196)
NG = 14                    # chunk groups (14 chunks of 128 each)
SCALE = 1.0 / 16.0         # 1/sqrt(256)

SLICES = [256, 256, 256, 256]
OFFS = [0, 256, 512, 768]
NSL = len(SLICES)
# attention-tile emission quanta per tile count of the previous slice
QA = {4: {2: 0, 5: 1, 8: 2, 11: 3}, 3: {2: 0, 5: 1, 8: 2}, 1: {2: 0}}

_CACHE = {}


def _build():
    import os

    STAGE = int(os.environ.get("KSTAGE", "0"))
    nc = bacc.Bacc("TRN2", target_bir_lowering=False, debug=False)

    xg_d = [
        nc.dram_tensor(f"xg{s}", [NG, 128, NG, cs], BF16, kind="ExternalInput")
        for s, cs in enumerate(SLICES)
    ]
    wg = nc.dram_tensor("wg", [NG, 128, NG, 128], BF16, kind="ExternalInput")
    f1g = nc.dram_tensor("f1g", [128, 4, R], BF16, kind="ExternalInput")
    wenc1 = nc.dram_tensor("wenc1", [128, 4, 128], BF16, kind="ExternalInput")
    b1 = nc.dram_tensor("b1", [DH, 1], F32, kind="ExternalInput")
    b2 = nc.dram_tensor("b2", [DH, 1], F32, kind="ExternalInput")
    waff = nc.dram_tensor("waff", [128, 2, 128], BF16, kind="ExternalInput")
    wca = nc.dram_tensor("wca", [128, 8, 128], BF16, kind="ExternalInput")
    wa = nc.dram_tensor("wa", [128, 128], BF16, kind="ExternalInput")
    wh = nc.dram_tensor("wh", [128, 2, 64], BF16, kind="ExternalInput")
    wreg = nc.dram_tensor("wreg", [128, 2, 2], BF16, kind="ExternalInput")
    creg = nc.dram_tensor("creg", [2, 1], F32, kind="ExternalInput")
    ident = nc.dram_tensor("ident", [128, 128], BF16, kind="ExternalInput")

    outs = nc.dram_tensor("outs", [2, R], F32, kind="ExternalOutput")

    from contextlib import ExitStack

    with tile.TileContext(nc) as tc:
        with ExitStack() as stack:
            ec = stack.enter_context
            cpool = ec(tc.tile_pool(name="const", bufs=1))
            wpool = ec(tc.tile_pool(name="wred", bufs=14))
            xpool = ec(tc.tile_pool(name="xin", bufs=6))
            actpool = ec(tc.tile_pool(name="acts", bufs=4))
            rtpool = ec(tc.tile_pool(name="rowsT", bufs=4))
            gsbpool = ec(tc.tile_pool(name="gsb", bufs=2))
            attsbpool = ec(tc.tile_pool(name="attsb", bufs=5))
            htsbpool = ec(tc.tile_pool(name="htsb", bufs=2))
            outsbpool = ec(tc.tile_pool(name="outsb", bufs=4))
            encpool = ec(tc.tile_pool(name="enc_ps", bufs=2, space="PSUM"))
            attpool = ec(tc.tile_pool(name="att_ps", bufs=2, space="PSUM"))
            midpool = ec(tc.tile_pool(name="mid_ps", bufs=2, space="PSUM"))
            outpool = ec(tc.tile_pool(name="out_ps", bufs=2, space="PSUM"))

            # ---- constants / weights (loaded once; f1 first — the slice-0
            # aud matmuls are the earliest consumer of a scalar-queued DMA)
            f1_sb = cpool.tile([128, 4, R], BF16, name="f1_sb")
            nc.scalar.dma_start(f1_sb[:], f1g[:])
            wenc1_sb = cpool.tile([128, 4, 128], BF16, name="wenc1_sb")
            nc.scalar.dma_start(wenc1_sb[:], wenc1[:])
            ident_sb = cpool.tile([128, 128], BF16, name="ident_sb")
            nc.scalar.dma_start(ident_sb[:], ident[:])
            b1_sb = cpool.tile([DH, 1], F32, name="b1_sb")
            nc.scalar.dma_start(b1_sb[:], b1[:])
            b2_sb = cpool.tile([DH, 1], F32, name="b2_sb")
            nc.scalar.dma_start(b2_sb[:], b2[:])
            creg_sb = cpool.tile([2, 1], F32, name="creg_sb")
            nc.scalar.dma_start(creg_sb[:], creg[:])
            waff_sb = cpool.tile([128, 2, 128], BF16, name="waff_sb")
            nc.scalar.dma_start(waff_sb[:], waff[:])
            wca_sb = cpool.tile([128, 8, 128], BF16, name="wca_sb")
            nc.scalar.dma_start(wca_sb[:], wca[:])
            wa_sb = cpool.tile([128, 128], BF16, name="wa_sb")
            nc.scalar.dma_start(wa_sb[:], wa[:])
            wh_sb = cpool.tile([128, 2, 64], BF16, name="wh_sb")
            nc.scalar.dma_start(wh_sb[:], wh[:])
            wreg_sb = cpool.tile([128, 2, 2], BF16, name="wreg_sb")
            nc.scalar.dma_start(wreg_sb[:], wreg[:])
            final_sb = cpool.tile([2, R], F32, name="final_sb")

            # ---- PE warmup: dummy matmuls to trip the HAM clock gate ----
            for i in range(32):
                wp = attpool.tile([128, 128], F32, tag="attps", name=f"warm{i}")
                nc.tensor.matmul(wp[:], ident_sb[:], ident_sb[:],
                                 start=True, stop=True)
            # preload the scalar-engine tanh table
            warmt_sb = cpool.tile([128, 4], BF16, name="warmt_sb", tag="warmt")
            nc.scalar.activation(warmt_sb[:], ident_sb[:, 0:4], AF.Tanh)

            # long-lived avf tiles, zeroed once: rows 16..31 of each 32-row
            # sample group are never written again, so the block-diagonal
            # weights always multiply zeros (not uninitialized bf16).
            avf_glob = []
            for i in range(6):
                av = cpool.tile([128, 256], BF16, tag=f"avfg{i}",
                                name=f"avfg{i}")
                nc.vector.memset(av[:], 0.0)
                avf_glob.append(av)

            w_tiles = []
            state = {}  # per-slice live tiles

            def emit_avf_tr(sp, blk):
                """PE-transpose 128 activation rows of slice sp into rt."""
                st = state[sp]
                cs = SLICES[sp]
                bw = min(128, cs - 128 * blk)
                rt = rtpool.tile([128, 256], BF16, tag="rt",
                                 name=f"rt{sp}_{blk}")
                st["rt"][blk] = rt
                for bi, src in enumerate((st["audT"], st["visT"])):
                    tr_ps = midpool.tile([128, 128], BF16, tag="mid",
                                         name=f"tr{sp}_{blk}_{bi}")
                    nc.tensor.transpose(
                        tr_ps[0:bw, :],
                        src[:, 128 * blk:128 * blk + bw],
                        ident_sb[:],
                    )
                    nc.vector.tensor_copy(rt[0:bw, 128 * bi:128 * bi + 128],
                                          tr_ps[0:bw, :])

            def emit_regroup(sp, a, eng=None):
                """Partition-regroup 4 samples of rt into avf tile a."""
                st = state[sp]
                rt = st["rt"][a // 2]
                for q in range(4):
                    m = 4 * (a % 2) + q
                    (eng or nc.scalar).dma_start(
                        st["avf"][a][32 * q:32 * q + 16, :],
                        rt[16 * m:16 * m + 16, :],
                    )

            def emit_attn_tile(sp, a):
                """Attention/branch stage for 4 samples (avf tile a, slice sp)."""
                st = state[sp]
                avf_t = st["avf"][a]
                g_ps = midpool.tile([128, 256], F32, tag="mid", name=f"g{sp}_{a}")
                for bi in range(2):
                    nc.tensor.matmul(
                        g_ps[:, 128 * bi:128 * bi + 128],
                        waff_sb[:, bi, :],
                        avf_t[:, 128 * bi:128 * bi + 128],
                        start=True, stop=True,
                    )
                g_sb = gsbpool.tile([128, 256], BF16, tag="gsb",
                                    name=f"gsb{sp}_{a}")
                nc.vector.tensor_copy(g_sb[:], g_ps[:])

                att_sbs = []
                for q in range(4):
                    att_ps = attpool.tile([128, 512], F32, tag="attps",
                                          name=f"att{sp}_{a}_{q}")
                    for jh in range(2):
                        nc.tensor.matmul(
                            att_ps[:, 256 * jh:256 * jh + 256],
                            avf_t[32 * q:32 * q + 16, 128 * jh:128 * jh + 128],
                            g_sb[32 * q:32 * q + 16, :],
                            start=True, stop=True,
                            tile_position=(32 * q, 0),
                        )
                    asb = attsbpool.tile([128, 512], BF16, tag="attsb",
                                         name=f"asb{sp}_{a}_{q}")
                    nc.scalar.activation(asb[:], att_ps[:], AF.Tanh, scale=SCALE)
                    att_sbs.append(asb)

                ht_ps = midpool.tile([128, 256], F32, tag="mid",
                                     name=f"ht{sp}_{a}")
                for q in range(4):
                    for jh in range(2):
                        nc.tensor.matmul(
                            ht_ps[:],
                            wca_sb[:, 2 * q + jh, :],
                            att_sbs[q][:, 256 * jh:256 * jh + 256],
                            start=(q == 0 and jh == 0), stop=False,
                        )
                nc.tensor.matmul(ht_ps[:], wa_sb[:], avf_t[:],
                                 start=False, stop=True)
                ht_sb = htsbpool.tile([128, 256], BF16, tag="htsb",
                                      name=f"htsb{sp}_{a}")
                nc.vector.tensor_relu(ht_sb[:], ht_ps[:])

                out_ps = outpool.tile([128, 2, 64], F32, tag="o2",
                                      name=f"o{sp}_{a}")
                for bi in range(2):
                    nc.tensor.matmul(
                        out_ps[:, bi, :],
                        ht_sb[:, 128 * bi:128 * bi + 128],
                        wh_sb[:, bi, :],
                        start=True, stop=True,
                    )
                for bi, act in enumerate((st["audT"], st["visT"])):
                    if STAGE == 1:
                        nc.vector.tensor_copy(
                            st["outsb"][bi][:, 64 * a:64 * a + 64],
                            act[:, 64 * a:64 * a + 64],
                        )
                    else:
                        nc.vector.tensor_add(
                            st["outsb"][bi][:, 64 * a:64 * a + 64],
                            out_ps[:, bi, :],
                            act[:, 64 * a:64 * a + 64],
                        )

            def emit_regressor(sp):
                st = state[sp]
                cs, off = SLICES[sp], OFFS[sp]
                out2_ps = outpool.tile([2, 256], F32, tag="o2",
                                       name=f"out2{sp}")
                nc.tensor.matmul(out2_ps[:, 0:cs], wreg_sb[:, 0, :],
                                 st["outsb"][0][:, 0:cs], start=True, stop=False)
                nc.tensor.matmul(out2_ps[:, 0:cs], wreg_sb[:, 1, :],
                                 st["outsb"][1][:, 0:cs], start=False, stop=True)
                nc.scalar.activation(
                    final_sb[:, off:off + cs], out2_ps[:, 0:cs],
                    AF.Identity, bias=creg_sb[:],
                )
                nc.scalar.dma_start(outs[:, off:off + cs],
                                    final_sb[:, off:off + cs])

            filler_n = [0]

            def emit_filler(n):
                # dummy matmuls that bridge PE-idle windows so the HAM
                # clock gate stays at 2.4 GHz across slice boundaries
                for _ in range(n):
                    i = filler_n[0]
                    filler_n[0] += 1
                    fp = attpool.tile([128, 128], F32, tag="attps",
                                      name=f"fill{i}")
                    nc.tensor.matmul(fp[:], ident_sb[:], ident_sb[:],
                                     start=True, stop=True)

            def attn_quantum(sp, g):
                if sp < 0:
                    return
                nt = SLICES[sp] // 64
                nblk = (nt + 1) // 2
                if g < nblk:
                    emit_avf_tr(sp, g)
                if g < nt:
                    emit_regroup(sp, g)
                qa = QA[nt]
                if g in qa:
                    emit_attn_tile(sp, qa[g])
                elif g == 12:
                    emit_regressor(sp)

            tile_base = 0
            for s, cs in enumerate(SLICES):
                off = OFFS[s]
                enc_ps = encpool.tile([128, 512], F32, tag="enc",
                                      name=f"enc{s}")
                for g in range(NG):
                    if s == 0:
                        wt = wpool.tile([128, NG, 128], BF16, tag="w",
                                        name=f"wt{g}")
                        nc.sync.dma_start(wt[:], wg[g])
                        w_tiles.append(wt)
                    xk = xpool.tile([128, NG, cs], BF16, tag="xk",
                                    name=f"xk{s}_{g}")
                    nc.sync.dma_start(xk[:], xg_d[s][g])
                    for j in range(NG):
                        nc.tensor.matmul(
                            enc_ps[:, 0:cs],
                            w_tiles[g][:, j, :],
                            xk[:, j, :],
                            start=(g == 0 and j == 0),
                            stop=(g == NG - 1 and j == NG - 1),
                        )
                    if g == 4:
                        # aud encoder rides inside the enc accumulation group
                        # (start bit already consumed by vis chunk 0)
                        for c in range(4):
                            nc.tensor.matmul(
                                enc_ps[:, 256:256 + cs],
                                wenc1_sb[:, c, :],
                                f1_sb[:, c, off:off + cs],
                                start=False, stop=False,
                            )
                    attn_quantum(s - 1, g)

                # slice boundary: move encoder outputs to SBUF (bias fused)
                audT = actpool.tile([128, 256], BF16, tag="act",
                                    name=f"audT{s}")
                nc.scalar.activation(audT[:, 0:cs], enc_ps[:, 256:256 + cs],
                                     AF.Identity, bias=b1_sb[:])
                visT = actpool.tile([128, 256], BF16, tag="act",
                                    name=f"visT{s}")
                nc.scalar.activation(visT[:, 0:cs], enc_ps[:, 0:cs],
                                     AF.Identity, bias=b2_sb[:])
                nt = cs // 64
                avf = [avf_glob[(tile_base + a) % 6] for a in range(nt)]
                tile_base += nt
                outsb = [
                    outsbpool.tile([128, 256], BF16, tag="outsb",
                                   name=f"os{s}_{bi}")
                    for bi in range(2)
                ]
                state[s] = dict(audT=audT, visT=visT, avf=avf, outsb=outsb,
                                rt=[None, None])

            # tail: attention for the last slice (sync queue is free now)
            sp = NSL - 1
            nt_t = SLICES[sp] // 64
            for blk in range((nt_t + 1) // 2):
                emit_avf_tr(sp, blk)
            for a in range(nt_t):
                emit_regroup(sp, a, eng=nc.sync)
            for a in range(nt_t):
                emit_attn_tile(sp, a)
            emit_regressor(sp)

    nc.compile()
    return nc


def _prep_shared(inputs):
    f32 = np.float32
    bf = ml_dtypes.bfloat16
    W_enc1 = np.asarray(inputs["W_enc1"], f32)
    W_enc2 = np.asarray(inputs["W_enc2"], f32)
    W_red = np.asarray(inputs["W_red"], f32)
    W2r = W_enc2 @ W_red                                    # [128, 25088]
    b2v = W_enc2 @ np.asarray(inputs["b_red"], f32) + np.asarray(inputs["b_enc2"], f32)
    wv = (np.asarray(inputs["Wv2"], f32) @ np.asarray(inputs["Wv1"], f32))[0]
    cv = float((np.asarray(inputs["Wv2"], f32) @ np.asarray(inputs["bv1"], f32)
                + np.asarray(inputs["bv2"], f32))[0])
    wa_ = (np.asarray(inputs["Wa2"], f32) @ np.asarray(inputs["Wa1"], f32))[0]
    ca = float((np.asarray(inputs["Wa2"], f32) @ np.asarray(inputs["ba1"], f32)
                + np.asarray(inputs["ba2"], f32))[0])

    W_affa = np.asarray(inputs["W_affa"], f32)
    W_affv = np.asarray(inputs["W_affv"], f32)
    W_a = np.asarray(inputs["W_a"], f32)
    W_ca = np.asarray(inputs["W_ca"], f32)
    W_ha = np.asarray(inputs["W_ha"], f32)
    W_hv = np.asarray(inputs["W_hv"], f32)

    waff_bd = np.zeros((2, 128, 128), f32)
    for q in range(4):
        waff_bd[0, 32 * q:32 * q + 16, 32 * q:32 * q + 16] = W_affa.T
        waff_bd[1, 32 * q:32 * q + 16, 32 * q:32 * q + 16] = W_affv.T
    wca_bd = np.zeros((4, 2, 128, 128), f32)
    for q in range(4):
        for jh in range(2):
            wca_bd[q, jh, :, 32 * q:32 * q + 32] = W_ca[:, 128 * jh:128 * jh + 128].T
    wa_bd = np.zeros((128, 128), f32)
    for q in range(4):
        wa_bd[32 * q:32 * q + 16, 32 * q:32 * q + 32] = W_a.T
    wh_bd = np.zeros((2, 128, 64), f32)
    for q in range(4):
        wh_bd[0, 32 * q:32 * q + 32, 16 * q:16 * q + 16] = W_ha.T
        wh_bd[1, 32 * q:32 * q + 32, 16 * q:16 * q + 16] = W_hv.T

    wg = np.ascontiguousarray(
        W2r.T.reshape(NG, NG, 128, 128).transpose(0, 2, 1, 3)
    ).astype(bf)
    wenc1g = np.ascontiguousarray(
        W_enc1.T.reshape(4, 128, 128).transpose(1, 0, 2)
    ).astype(bf)

    shared = {
        "wg": wg,
        "wenc1": wenc1g,
        "b1": np.asarray(inputs["b_enc1"], f32).reshape(128, 1),
        "b2": b2v.reshape(128, 1),
        "waff": np.ascontiguousarray(waff_bd.transpose(1, 0, 2)).astype(bf),
        "wca": np.ascontiguousarray(
            wca_bd.reshape(8, 128, 128).transpose(1, 0, 2)
        ).astype(bf),
        "wa": wa_bd.astype(bf),
        "wh": np.ascontiguousarray(wh_bd.transpose(1, 0, 2)).astype(bf),
        "wreg": np.ascontiguousarray(
            np.stack(
                [np.stack([wv[:128], wa_[:128]], 1),
                 np.stack([wv[128:], wa_[128:]], 1)], 1
            )
        ).astype(bf),
        "creg": np.array([[cv], [ca]], f32),
        "ident": np.eye(128, dtype=f32).astype(bf),
    }
    return shared


def kernel(**inputs):
    if "nc" not in _CACHE:
        _CACHE["nc"] = _build()
    nc = _CACHE["nc"]

    bf = ml_dtypes.bfloat16
    shared = _prep_shared(inputs)

    f1 = np.asarray(inputs["f1_norm"], np.float32).reshape(B * T, DA)
    f2 = np.asarray(inputs["f2_norm"], np.float32).reshape(B * T, DV)

    in_maps = []
    for c in range(NCORES):
        rs = slice(c * R, (c + 1) * R)
        m = dict(shared)
        xT4 = f2[rs].T.reshape(NG, NG, 128, R)          # [g, j, p, r]
        for s, cs in enumerate(SLICES):
            off = OFFS[s]
            m[f"xg{s}"] = np.ascontiguousarray(
                xT4[:, :, :, off:off + cs].transpose(0, 2, 1, 3)
            ).astype(bf)
        m["f1g"] = np.ascontiguousarray(
            f1[rs].T.reshape(4, 128, R).transpose(1, 0, 2)
        ).astype(bf)
        in_maps.append(m)

    import os

    res = bass_utils.run_bass_kernel_spmd(
        nc,
        in_maps,
        core_ids=list(range(NCORES)),
        trace=bool(os.environ.get("KERNEL_TRACE")),
    )
    _CACHE["last_results"] = res

    vouts = np.concatenate(
        [r["outs"][0].reshape(S, T) for r in res.results], axis=0
    ).astype(np.float32)
    aouts = np.concatenate(
        [r["outs"][1].reshape(S, T) for r in res.results], axis=0
    ).astype(np.float32)
    return vouts, aouts


# revision 21
# speedup vs baseline: 1.0742x; 1.0556x over previous
"""Trainium2 Bass kernel for nn_CAM_6949257085456.

Data-parallel over batch: 8 cores x 64 samples (1024 activation rows each).
Per core the rows are processed as 5 pipeline slices (256/256/256/192/64):

  stream(s):  large fully-contiguous x DMAs feed the folded vis encoder
              matmul  visT = (W_enc2 @ W_red) @ x  (196 k-chunks, N=cs)
  attn(s-1):  the attention/branch stage of the previous slice is emitted
              interleaved between the stream's matmul groups so the PE stays
              dense-busy (keeps the HAM clock-gate at 2.4 GHz) while DMA paces.
              The last slice is small (64 rows) so the un-overlapped tail is
              one attention tile.

Attention restructure vs the obvious per-sample loop: 4 samples live in one
128-partition tile at 32-row pitch, and the per-sample 16/32-contraction
linears (W_aff / W_a / W_ca / W_h) are folded into block-diagonal 128-row
weights on the host, so each stage is a single full-K matmul (or a
tile_position row-strip matmul for the data*data attention product).

Host-side algebraic folds (exact in fp32):
  - vis path: X @ W_red.T @ W_enc2.T == X @ (W_enc2 @ W_red).T
  - regressors: feats@Wv1.T@Wv2.T == feats @ (Wv2@Wv1).T
Everything fed to the chip is bf16 (fp32 PSUM accumulation).
"""
import sys

if "/opt/trn_rl_repo" not in sys.path:
    sys.path.insert(0, "/opt/trn_rl_repo")

import numpy as np
import ml_dtypes

import concourse.bacc as bacc
import concourse.bass as bass
import concourse.mybir as mybir
import concourse.tile as tile
from concourse import bass_utils

BF16 = mybir.dt.bfloat16
F32 = mybir.dt.float32
AF = mybir.ActivationFunctionType

B, T, DA, DV, DH = 512, 16, 512, 25088, 128
NCORES = 8
S = B // NCORES            # samples per core (64)
R = S * T                  # rows per core (1024)
KC = DV // 128             # contraction chunks (196)
NG = 14                    # chunk groups (14 chunks of 128 each)
SCALE = 1.0 / 16.0         # 1/sqrt(256)

SLICES = [256, 256, 256, 256]
OFFS = [0, 256, 512, 768]
NSL = len(SLICES)
# attention-tile emission quanta per tile count of the previous slice
QA = {4: {2: 0, 5: 1, 8: 2, 11: 3}, 3: {2: 0, 5: 1, 8: 2}, 1: {2: 0}}

_CACHE = {}


def _build():
    import os

    STAGE = int(os.environ.get("KSTAGE", "0"))
    nc = bacc.Bacc("TRN2", target_bir_lowering=False, debug=False)

    xg_d = [
        nc.dram_tensor(f"xg{s}", [NG, 128, NG, cs], BF16, kind="ExternalInput")
        for s, cs in enumerate(SLICES)
    ]
    wg = nc.dram_tensor("wg", [NG, 128, NG, 128], BF16, kind="ExternalInput")
    f1g = nc.dram_tensor("f1g", [128, 4, R], BF16, kind="ExternalInput")
    wenc1 = nc.dram_tensor("wenc1", [128, 4, 128], BF16, kind="ExternalInput")
    b1 = nc.dram_tensor("b1", [DH, 1], F32, kind="ExternalInput")
    b2 = nc.dram_tensor("b2", [DH, 1], F32, kind="ExternalInput")
    waff = nc.dram_tensor("waff", [128, 2, 128], BF16, kind="ExternalInput")
    wca = nc.dram_tensor("wca", [128, 8, 128], BF16, kind="ExternalInput")
    wa = nc.dram_tensor("wa", [128, 128], BF16, kind="ExternalInput")
    wh = nc.dram_tensor("wh", [128, 2, 64], BF16, kind="ExternalInput")
    wreg = nc.dram_tensor("wreg", [128, 2, 2], BF16, kind="ExternalInput")
    creg = nc.dram_tensor("creg", [2, 1], F32, kind="ExternalInput")
    ident = nc.dram_tensor("ident", [128, 128], BF16, kind="ExternalInput")

    outs = nc.dram_tensor("outs", [2, R], F32, kind="ExternalOutput")

    from contextlib import ExitStack

    with tile.TileContext(nc) as tc:
        with ExitStack() as stack:
            ec = stack.enter_context
            cpool = ec(tc.tile_pool(name="const", bufs=1))
            wpool = ec(tc.tile_pool(name="wred", bufs=14))
            xpool = ec(tc.tile_pool(name="xin", bufs=8))
            actpool = ec(tc.tile_pool(name="acts", bufs=4))
            rtpool = ec(tc.tile_pool(name="rowsT", bufs=4))
            gsbpool = ec(tc.tile_pool(name="gsb", bufs=2))
            attsbpool = ec(tc.tile_pool(name="attsb", bufs=5))
            htsbpool = ec(tc.tile_pool(name="htsb", bufs=2))
            outsbpool = ec(tc.tile_pool(name="outsb", bufs=4))
            encpool = ec(tc.tile_pool(name="enc_ps", bufs=2, space="PSUM"))
            attpool = ec(tc.tile_pool(name="att_ps", bufs=2, space="PSUM"))
            midpool = ec(tc.tile_pool(name="mid_ps", bufs=2, space="PSUM"))
            outpool = ec(tc.tile_pool(name="out_ps", bufs=2, space="PSUM"))

            # ---- constants / weights (loaded once; f1 first — the slice-0
            # aud matmuls are the earliest consumer of a scalar-queued DMA)
            f1_sb = cpool.tile([128, 4, R], BF16, name="f1_sb")
            nc.scalar.dma_start(f1_sb[:], f1g[:])
            wenc1_sb = cpool.tile([128, 4, 128], BF16, name="wenc1_sb")
            nc.scalar.dma_start(wenc1_sb[:], wenc1[:])
            ident_sb = cpool.tile([128, 128], BF16, name="ident_sb")
            nc.scalar.dma_start(ident_sb[:], ident[:])
            b1_sb = cpool.tile([DH, 1], F32, name="b1_sb")
            nc.scalar.dma_start(b1_sb[:], b1[:])
            b2_sb = cpool.tile([DH, 1], F32, name="b2_sb")
            nc.scalar.dma_start(b2_sb[:], b2[:])
            creg_sb = cpool.tile([2, 1], F32, name="creg_sb")
            nc.scalar.dma_start(creg_sb[:], creg[:])
            waff_sb = cpool.tile([128, 2, 128], BF16, name="waff_sb")
            nc.scalar.dma_start(waff_sb[:], waff[:])
            wca_sb = cpool.tile([128, 8, 128], BF16, name="wca_sb")
            nc.scalar.dma_start(wca_sb[:], wca[:])
            wa_sb = cpool.tile([128, 128], BF16, name="wa_sb")
            nc.scalar.dma_start(wa_sb[:], wa[:])
            wh_sb = cpool.tile([128, 2, 64], BF16, name="wh_sb")
            nc.scalar.dma_start(wh_sb[:], wh[:])
            wreg_sb = cpool.tile([128, 2, 2], BF16, name="wreg_sb")
            nc.scalar.dma_start(wreg_sb[:], wreg[:])
            final_sb = cpool.tile([2, R], F32, name="final_sb")

            # ---- PE warmup: dummy matmuls to trip the HAM clock gate ----
            for i in range(32):
                wp = attpool.tile([128, 128], F32, tag="attps", name=f"warm{i}")
                nc.tensor.matmul(wp[:], ident_sb[:], ident_sb[:],
                                 start=True, stop=True)
            # preload the scalar-engine tanh table
            warmt_sb = cpool.tile([128, 4], BF16, name="warmt_sb", tag="warmt")
            nc.scalar.activation(warmt_sb[:], ident_sb[:, 0:4], AF.Tanh)

            # long-lived avf tiles, zeroed once: rows 16..31 of each 32-row
            # sample group are never written again, so the block-diagonal
            # weights always multiply zeros (not uninitialized bf16).
            avf_glob = []
            for i in range(6):
                av = cpool.tile([128, 256], BF16, tag=f"avfg{i}",
                                name=f"avfg{i}")
                nc.vector.memset(av[:], 0.0)
                avf_glob.append(av)

            w_tiles = []
            state = {}  # per-slice live tiles

            def emit_avf_tr(sp, blk):
                """PE-transpose 128 activation rows of slice sp into rt."""
                st = state[sp]
                cs = SLICES[sp]
                bw = min(128, cs - 128 * blk)
                rt = rtpool.tile([128, 256], BF16, tag="rt",
                                 name=f"rt{sp}_{blk}")
                st["rt"][blk] = rt
                for bi, src in enumerate((st["audT"], st["visT"])):
                    tr_ps = midpool.tile([128, 128], BF16, tag="mid",
                                         name=f"tr{sp}_{blk}_{bi}")
                    nc.tensor.transpose(
                        tr_ps[0:bw, :],
                        src[:, 128 * blk:128 * blk + bw],
                        ident_sb[:],
                    )
                    nc.vector.tensor_copy(rt[0:bw, 128 * bi:128 * bi + 128],
                                          tr_ps[0:bw, :])

            def emit_regroup(sp, a, eng=None):
                """Partition-regroup 4 samples of rt into avf tile a."""
                st = state[sp]
                rt = st["rt"][a // 2]
                for q in range(4):
                    m = 4 * (a % 2) + q
                    (eng or nc.scalar).dma_start(
                        st["avf"][a][32 * q:32 * q + 16, :],
                        rt[16 * m:16 * m + 16, :],
                    )

            def emit_attn_tile(sp, a):
                """Attention/branch stage for 4 samples (avf tile a, slice sp)."""
                st = state[sp]
                avf_t = st["avf"][a]
                g_ps = midpool.tile([128, 256], F32, tag="mid", name=f"g{sp}_{a}")
                for bi in range(2):
                    nc.tensor.matmul(
                        g_ps[:, 128 * bi:128 * bi + 128],
                        waff_sb[:, bi, :],
                        avf_t[:, 128 * bi:128 * bi + 128],
                        start=True, stop=True,
                    )
                g_sb = gsbpool.tile([128, 256], BF16, tag="gsb",
                                    name=f"gsb{sp}_{a}")
                nc.vector.tensor_copy(g_sb[:], g_ps[:])

                att_sbs = []
                for q in range(4):
                    att_ps = attpool.tile([128, 512], F32, tag="attps",
                                          name=f"att{sp}_{a}_{q}")
                    for jh in range(2):
                        nc.tensor.matmul(
                            att_ps[:, 256 * jh:256 * jh + 256],
                            avf_t[32 * q:32 * q + 16, 128 * jh:128 * jh + 128],
                            g_sb[32 * q:32 * q + 16, :],
                            start=True, stop=True,
                            tile_position=(32 * q, 0),
                        )
                    asb = attsbpool.tile([128, 512], BF16, tag="attsb",
                                         name=f"asb{sp}_{a}_{q}")
                    nc.scalar.activation(asb[:], att_ps[:], AF.Tanh, scale=SCALE)
                    att_sbs.append(asb)

                ht_ps = midpool.tile([128, 256], F32, tag="mid",
                                     name=f"ht{sp}_{a}")
                for q in range(4):
                    for jh in range(2):
                        nc.tensor.matmul(
                            ht_ps[:],
                            wca_sb[:, 2 * q + jh, :],
                            att_sbs[q][:, 256 * jh:256 * jh + 256],
                            start=(q == 0 and jh == 0), stop=False,
                        )
                nc.tensor.matmul(ht_ps[:], wa_sb[:], avf_t[:],
                                 start=False, stop=True)
                ht_sb = htsbpool.tile([128, 256], BF16, tag="htsb",
                                      name=f"htsb{sp}_{a}")
                nc.vector.tensor_relu(ht_sb[:], ht_ps[:])

                out_ps = outpool.tile([128, 2, 64], F32, tag="o2",
                                      name=f"o{sp}_{a}")
                for bi in range(2):
                    nc.tensor.matmul(
                        out_ps[:, bi, :],
                        ht_sb[:, 128 * bi:128 * bi + 128],
                        wh_sb[:, bi, :],
                        start=True, stop=True,
                    )
                for bi, act in enumerate((st["audT"], st["visT"])):
                    if STAGE == 1:
                        nc.vector.tensor_copy(
                            st["outsb"][bi][:, 64 * a:64 * a + 64],
                            act[:, 64 * a:64 * a + 64],
                        )
                    else:
                        nc.vector.tensor_add(
                            st["outsb"][bi][:, 64 * a:64 * a + 64],
                            out_ps[:, bi, :],
                            act[:, 64 * a:64 * a + 64],
                        )

            def emit_regressor(sp):
                st = state[sp]
                cs, off = SLICES[sp], OFFS[sp]
                out2_ps = outpool.tile([2, 256], F32, tag="o2",
                                       name=f"out2{sp}")
                nc.tensor.matmul(out2_ps[:, 0:cs], wreg_sb[:, 0, :],
                                 st["outsb"][0][:, 0:cs], start=True, stop=False)
                nc.tensor.matmul(out2_ps[:, 0:cs], wreg_sb[:, 1, :],
                                 st["outsb"][1][:, 0:cs], start=False, stop=True)
                nc.scalar.activation(
                    final_sb[:, off:off + cs], out2_ps[:, 0:cs],
                    AF.Identity, bias=creg_sb[:],
                )
                nc.scalar.dma_start(outs[:, off:off + cs],
                                    final_sb[:, off:off + cs])

            filler_n = [0]

            def emit_filler(n):
                # dummy matmuls that bridge PE-idle windows so the HAM
                # clock gate stays at 2.4 GHz across slice boundaries
                for _ in range(n):
                    i = filler_n[0]
                    filler_n[0] += 1
                    fp = attpool.tile([128, 128], F32, tag="attps",
                                      name=f"fill{i}")
                    nc.tensor.matmul(fp[:], ident_sb[:], ident_sb[:],
                                     start=True, stop=True)

            def attn_quantum(sp, g):
                if sp < 0:
                    return
                nt = SLICES[sp] // 64
                nblk = (nt + 1) // 2
                if g < nblk:
                    emit_avf_tr(sp, g)
                if g < nt:
                    emit_regroup(sp, g)
                qa = QA[nt]
                if g in qa:
                    emit_attn_tile(sp, qa[g])
                elif g == 12:
                    emit_regressor(sp)

            tile_base = 0
            for s, cs in enumerate(SLICES):
                off = OFFS[s]
                enc_ps = encpool.tile([128, 512], F32, tag="enc",
                                      name=f"enc{s}")
                for g in range(NG):
                    if s == 0:
                        wt = wpool.tile([128, NG, 128], BF16, tag="w",
                                        name=f"wt{g}")
                        nc.sync.dma_start(wt[:], wg[g])
                        w_tiles.append(wt)
                    xk = xpool.tile([128, NG, cs], BF16, tag="xk",
                                    name=f"xk{s}_{g}")
                    nc.sync.dma_start(xk[:], xg_d[s][g])
                    for j in range(NG):
                        nc.tensor.matmul(
                            enc_ps[:, 0:cs],
                            w_tiles[g][:, j, :],
                            xk[:, j, :],
                            start=(g == 0 and j == 0),
                            stop=(g == NG - 1 and j == NG - 1),
                        )
                    if g == 4:
                        # aud encoder rides inside the enc accumulation group
                        # (start bit already consumed by vis chunk 0)
                        for c in range(4):
                            nc.tensor.matmul(
                                enc_ps[:, 256:256 + cs],
                                wenc1_sb[:, c, :],
                                f1_sb[:, c, off:off + cs],
                                start=False, stop=False,
                            )
                    attn_quantum(s - 1, g)

                # slice boundary: move encoder outputs to SBUF (bias fused)
                audT = actpool.tile([128, 256], BF16, tag="act",
                                    name=f"audT{s}")
                nc.scalar.activation(audT[:, 0:cs], enc_ps[:, 256:256 + cs],
                                     AF.Identity, bias=b1_sb[:])
                visT = actpool.tile([128, 256], BF16, tag="act",
                                    name=f"visT{s}")
                nc.scalar.activation(visT[:, 0:cs], enc_ps[:, 0:cs],
                                     AF.Identity, bias=b2_sb[:])
                nt = cs // 64
                avf = [avf_glob[(tile_base + a) % 6] for a in range(nt)]
                tile_base += nt
                outsb = [
                    outsbpool.tile([128, 256], BF16, tag="outsb",
                                   name=f"os{s}_{bi}")
                    for bi in range(2)
                ]
                state[s] = dict(audT=audT, visT=visT, avf=avf, outsb=outsb,
                                rt=[None, None])

            # tail: attention for the last slice (sync queue is free now)
            sp = NSL - 1
            nt_t = SLICES[sp] // 64
            for blk in range((nt_t + 1) // 2):
                emit_avf_tr(sp, blk)
            for a in range(nt_t):
                emit_regroup(sp, a, eng=nc.sync)
            for a in range(nt_t):
                emit_attn_tile(sp, a)
            emit_regressor(sp)

    nc.compile()
    return nc


def _prep_shared(inputs):
    f32 = np.float32
    bf = ml_dtypes.bfloat16
    W_enc1 = np.asarray(inputs["W_enc1"], f32)
    W_enc2 = np.asarray(inputs["W_enc2"], f32)
    W_red = np.asarray(inputs["W_red"], f32)
    W2r = W_enc2 @ W_red                                    # [128, 25088]
    b2v = W_enc2 @ np.asarray(inputs["b_red"], f32) + np.asarray(inputs["b_enc2"], f32)
    wv = (np.asarray(inputs["Wv2"], f32) @ np.asarray(inputs["Wv1"], f32))[0]
    cv = float((np.asarray(inputs["Wv2"], f32) @ np.asarray(inputs["bv1"], f32)
                + np.asarray(inputs["bv2"], f32))[0])
    wa_ = (np.asarray(inputs["Wa2"], f32) @ np.asarray(inputs["Wa1"], f32))[0]
    ca = float((np.asarray(inputs["Wa2"], f32) @ np.asarray(inputs["ba1"], f32)
                + np.asarray(inputs["ba2"], f32))[0])

    W_affa = np.asarray(inputs["W_affa"], f32)
    W_affv = np.asarray(inputs["W_affv"], f32)
    W_a = np.asarray(inputs["W_a"], f32)
    W_ca = np.asarray(inputs["W_ca"], f32)
    W_ha = np.asarray(inputs["W_ha"], f32)
    W_hv = np.asarray(inputs["W_hv"], f32)

    waff_bd = np.zeros((2, 128, 128), f32)
    for q in range(4):
        waff_bd[0, 32 * q:32 * q + 16, 32 * q:32 * q + 16] = W_affa.T
        waff_bd[1, 32 * q:32 * q + 16, 32 * q:32 * q + 16] = W_affv.T
    wca_bd = np.zeros((4, 2, 128, 128), f32)
    for q in range(4):
        for jh in range(2):
            wca_bd[q, jh, :, 32 * q:32 * q + 32] = W_ca[:, 128 * jh:128 * jh + 128].T
    wa_bd = np.zeros((128, 128), f32)
    for q in range(4):
        wa_bd[32 * q:32 * q + 16, 32 * q:32 * q + 32] = W_a.T
    wh_bd = np.zeros((2, 128, 64), f32)
    for q in range(4):
        wh_bd[0, 32 * q:32 * q + 32, 16 * q:16 * q + 16] = W_ha.T
        wh_bd[1, 32 * q:32 * q + 32, 16 * q:16 * q + 16] = W_hv.T

    wg = np.ascontiguousarray(
        W2r.T.reshape(NG, NG, 128, 128).transpose(0, 2, 1, 3)
    ).astype(bf)
    wenc1g = np.ascontiguousarray(
        W_enc1.T.reshape(4, 128, 128).transpose(1, 0, 2)
    ).astype(bf)

    shared = {
        "wg": wg,
        "wenc1": wenc1g,
        "b1": np.asarray(inputs["b_enc1"], f32).reshape(128, 1),
        "b2": b2v.reshape(128, 1),
        "waff": np.ascontiguousarray(waff_bd.transpose(1, 0, 2)).astype(bf),
        "wca": np.ascontiguousarray(
            wca_bd.reshape(8, 128, 128).transpose(1, 0, 2)
        ).astype(bf),
        "wa": wa_bd.astype(bf),
        "wh": np.ascontiguousarray(wh_bd.transpose(1, 0, 2)).astype(bf),
        "wreg": np.ascontiguousarray(
            np.stack(
                [np.stack([wv[:128], wa_[:128]], 1),
                 np.stack([wv[128:], wa_[128:]], 1)], 1
            )
        ).astype(bf),
        "creg": np.array([[cv], [ca]], f32),
        "ident": np.eye(128, dtype=f32).astype(bf),
    }
    return shared


def kernel(**inputs):
    if "nc" not in _CACHE:
        _CACHE["nc"] = _build()
    nc = _CACHE["nc"]

    bf = ml_dtypes.bfloat16
    shared = _prep_shared(inputs)

    f1 = np.asarray(inputs["f1_norm"], np.float32).reshape(B * T, DA)
    f2 = np.asarray(inputs["f2_norm"], np.float32).reshape(B * T, DV)

    in_maps = []
    for c in range(NCORES):
        rs = slice(c * R, (c + 1) * R)
        m = dict(shared)
        xT4 = f2[rs].T.reshape(NG, NG, 128, R)          # [g, j, p, r]
        for s, cs in enumerate(SLICES):
            off = OFFS[s]
            m[f"xg{s}"] = np.ascontiguousarray(
                xT4[:, :, :, off:off + cs].transpose(0, 2, 1, 3)
            ).astype(bf)
        m["f1g"] = np.ascontiguousarray(
            f1[rs].T.reshape(4, 128, R).transpose(1, 0, 2)
        ).astype(bf)
        in_maps.append(m)

    import os

    res = bass_utils.run_bass_kernel_spmd(
        nc,
        in_maps,
        core_ids=list(range(NCORES)),
        trace=bool(os.environ.get("KERNEL_TRACE")),
    )
    _CACHE["last_results"] = res

    vouts = np.concatenate(
        [r["outs"][0].reshape(S, T) for r in res.results], axis=0
    ).astype(np.float32)
    aouts = np.concatenate(
        [r["outs"][1].reshape(S, T) for r in res.results], axis=0
    ).astype(np.float32)
    return vouts, aouts
